# revision 1
# baseline (speedup 1.0000x reference)
"""DGCNN (nn_DGCNN_40097814675951) Trainium2 Bass kernel.

kernel(x, params) -> [4, 40] logits, numerically matching the jax reference.

Distribution: 8 NeuronCores; cores (2b, 2b+1) handle batch b with a
query-split (1024 points each). Per-layer kNN (PE Gram + DVE max8 top-20),
neighbor max-aggregation via per-partition indirect-DMA row gathers of
fused z-tables, pairwise AllGather to exchange computed feature halves,
classifier head computed redundantly per pair.
"""
import sys
import numpy as np

sys.path.insert(0, "/opt/trn_rl_repo")

import concourse.bass as bass
import concourse.bacc as bacc
import concourse.tile as tile
from concourse import mybir
from concourse import bass_utils
from concourse.masks import make_identity

dt = mybir.dt
AF = mybir.ActivationFunctionType

P = 128
N = 2048
NQ = 1024
NQT = 8
NJT = 16
NEG = -3.0e38
CH = [64, 64, 128, 256]
PAIRS = [[0, 1], [2, 3], [4, 5], [6, 7]]
B = 4
BN_EPS = 1e-5
EMB = 1024

WSHAPES = [
    ("Az0", [3, 64]), ("CzA0", [6, 64]), ("CzB0", [1, 64]), ("WrT0", [6, 64]),
    ("Az1", [64, 64]), ("CzA1", [64, 64]), ("CzB1", [1, 64]), ("WrT1", [64, 64]),
    ("Az2", [64, 128]), ("CzA2", [64, 128]), ("CzB2", [1, 128]), ("WrT2", [64, 128]),
    ("Az3", [128, 256]), ("CzA3", [128, 256]), ("CzB3", [1, 256]), ("WrT3", [128, 256]),
    ("rhs_zw", [3, 512]), ("rhs_cwA", [3, 512]), ("rhs_cwB", [1, 512]),
    ("spre_pack", [128, 8]), ("bpre_pack", [128, 8]),
    ("s1_pack", [128, 4]), ("b1_pack", [128, 4]),
    ("w2T_pack", [128, 4, 256]), ("s2_pack", [128, 2]), ("b2_pack", [128, 2]),
    ("w3T_pack", [128, 2, 40]), ("b3_col", [40, 1]),
]
WSTREAM = [("wpreT_pack", [128, 5, 1024]), ("w1T_pack", [128, 16, 512])]


# ======================================================================
# device program
# ======================================================================
def build_nc():
    nc = bacc.Bacc("TRN2", target_bir_lowering=False, debug=False, num_devices=8)

    xT_d = nc.dram_tensor("xT", [6, N], dt.float32, kind="ExternalInput")
    wd = {}
    for name, shape in WSHAPES + WSTREAM:
        wd[name] = nc.dram_tensor(name, shape, dt.float32, kind="ExternalInput")
    logits_d = nc.dram_tensor("logits", [40, 1], dt.float32, kind="ExternalOutput")

    with tile.TileContext(nc) as tc:
        consts = tc.alloc_tile_pool(name="consts", bufs=1)
        feat = tc.alloc_tile_pool(name="feat", bufs=1)
        dram = tc.alloc_tile_pool(name="dram", bufs=1, space="DRAM")
        psum = tc.alloc_tile_pool(name="psum", bufs=1, space="PSUM")
        work = tc.alloc_tile_pool(name="work", bufs=1)

        xT = consts.tile([6, N], dt.float32)
        nc.sync.dma_start(out=xT, in_=xT_d[:, :])
        xyzT = xT[0:3, :]
        ones_row = consts.tile([1, N], dt.float32)
        nc.vector.memset(ones_row, 1.0)
        onesC = consts.tile([128, 1], dt.float32)
        nc.vector.memset(onesC, 1.0)
        ident = consts.tile([128, 128], dt.float32)
        make_identity(nc, ident[:])
        W = {}
        for name, shape in WSHAPES:
            W[name] = consts.tile(list(shape), dt.float32, tag=f"w_{name}", name=f"w_{name}")
            nc.sync.dma_start(out=W[name], in_=wd[name][:])

        qoff = (nc.scalar.partition_id() % 2) * NQ
        qoff_v = (nc.vector.partition_id() % 2) * NQ

        X1T = feat.tile([64, N], dt.float32)
        X2T = feat.tile([64, N], dt.float32)
        X3T = feat.tile([128, N], dt.float32)
        X4T = feat.tile([128, 2, NQ], dt.float32)
        Wall = feat.tile([128, NQT, 512], dt.float32)
        negxx = feat.tile([1, N], dt.float32)
        negxxq = feat.tile([1, NQ], dt.float32)
        xq = feat.tile([6, NQ], dt.float32)
        Xq = {1: feat.tile([64, NQ], dt.float32, name="X1q"),
              2: feat.tile([64, NQ], dt.float32, name="X2q"),
              3: feat.tile([128, NQ], dt.float32, name="X3q")}
        x4keep = [feat.tile([128, 256], dt.float32, tag=f"x4_{i}", name=f"x4_{i}")
                  for i in range(NQT)]

        z0w_dr = dram.tile([N, 576], dt.float32, tag="z0w", name="z0w")
        z_dr = {1: dram.tile([N, 64], dt.float32, tag="z1", name="z1"),
                2: dram.tile([N, 128], dt.float32, tag="z2", name="z2"),
                3: dram.tile([N, 256], dt.float32, tag="z3", name="z3")}
        agin = {l: dram.tile([NQ, CH[l - 1]], dt.float32, tag=f"agin{l}", name=f"agin{l}")
                for l in (1, 2, 3)}
        agout = {l: dram.tile([N, CH[l - 1]], dt.float32, tag=f"agout{l}", name=f"agout{l}")
                 for l in (1, 2, 3)}
        pagin = dram.tile([128, 16], dt.float32, tag="pagin", name="pagin")
        pagout = dram.tile([256, 16], dt.float32, tag="pagout", name="pagout")

        def ps_gram():
            return psum.tile([128, 512], dt.float32, tag="psgram", name="psgram", bufs=2)

        def ps_z():
            return psum.tile([128, 256], dt.float32, tag="psz", name="psz", bufs=2)

        def ps_c():
            return psum.tile([128, 256], dt.float32, tag="psc", name="psc", bufs=1)

        def ps_r():
            return psum.tile([128, 256], dt.float32, tag="psr", name="psr", bufs=1)

        def ps_tr():
            return psum.tile([128, 128], dt.float32, tag="pstr", name="pstr", bufs=1)

        def ps_small():
            return psum.tile([128, 512], dt.float32, tag="pssmall", name="pssmall", bufs=1)

        def compute_negxx(XT_ap, C):
            sq = work.tile([128, N], dt.float32, tag="pd", bufs=2)
            nc.vector.tensor_mul(sq[0:C, :], XT_ap, XT_ap)
            for jc in range(4):
                ps = ps_small()
                nc.tensor.matmul(ps[0:1, :], lhsT=onesC[0:C, 0:1],
                                 rhs=sq[0:C, bass.ts(jc, 512)], start=True, stop=True)
                nc.scalar.activation(negxx[0:1, bass.ts(jc, 512)], ps[0:1, :], AF.Copy,
                                     scale=-0.5)
            nc.vector.tensor_copy(negxxq, negxx[0:1, bass.ds(qoff_v, NQ)])

        def gram_topk(XT_ap, XqT_ap, idx_all):
            for i in range(NQT):
                qs = bass.ts(i, P)
                pd = work.tile([128, N], dt.float32, tag="pd", bufs=2)
                for jc in range(4):
                    ps = ps_gram()
                    js = bass.ts(jc, 512)
                    nc.tensor.matmul(ps, lhsT=XqT_ap[:, qs], rhs=XT_ap[:, js],
                                     start=True, stop=False)
                    nc.tensor.matmul(ps, lhsT=ones_row[0:1, qs],
                                     rhs=negxx[0:1, js], start=False, stop=False)
                    nc.tensor.matmul(ps, lhsT=negxxq[0:1, qs],
                                     rhs=ones_row[0:1, js], start=False, stop=True)
                    nc.scalar.copy(pd[:, js], ps)
                vals = work.tile([128, 24], dt.float32, tag="tkvals", bufs=2)
                for r in range(3):
                    vs = vals[:, r * 8:(r + 1) * 8]
                    nc.vector.max(out=vs, in_=pd)
                    nc.vector.max_index(out=idx_all[:, i, r * 8:(r + 1) * 8],
                                        in_max=vs, in_values=pd)
                    if r < 2:
                        nc.vector.match_replace(out=pd, in_to_replace=vs, in_values=pd,
                                                imm_value=NEG)

        def gather_kmax(z_ap, CW, idx_all, i):
            km = work.tile([128, 576], dt.float32, tag="gkm", bufs=2)
            for h in range(4):
                g = work.tile([128, 5, 576], dt.float32, tag="g", bufs=2)
                for k in range(5):
                    nc.gpsimd.indirect_dma_start(
                        out=g[:, k, 0:CW], out_offset=None,
                        in_=z_ap,
                        in_offset=bass.IndirectOffsetOnAxis(
                            ap=idx_all[:, i, h * 5 + k:h * 5 + k + 1], axis=0),
                    )
                red = work.tile([128, 576], dt.float32, tag="gred", bufs=2)
                nc.vector.reduce_max(red[:, 0:CW],
                                     g[:, :, 0:CW].rearrange("p k c -> p c k"),
                                     axis=mybir.AxisListType.X)
                if h == 0:
                    nc.vector.tensor_copy(km[:, 0:CW], red[:, 0:CW])
                else:
                    nc.vector.tensor_tensor(out=km[:, 0:CW], in0=km[:, 0:CW],
                                            in1=red[:, 0:CW], op=mybir.AluOpType.max)
            return km

        # ===== layer 0 =====
        idx0_all = consts.tile([128, NQT, 24], dt.uint32, tag="idx0")
        nc.scalar.copy(xq, xT[:, bass.ds(qoff, NQ)])
        xyzq = xq[0:3, :]
        compute_negxx(xyzT, 3)
        gram_topk(xyzT, xyzq, idx0_all)

        for j in range(NJT):
            ps64 = ps_z()
            ps512 = ps_gram()
            ljs = xyzT[:, bass.ts(j, P)]
            nc.tensor.matmul(ps64[:, 0:64], lhsT=ljs, rhs=W["Az0"], start=True, stop=True)
            nc.tensor.matmul(ps512, lhsT=ljs, rhs=W["rhs_zw"], start=True, stop=True)
            zrow = work.tile([128, 576], dt.float32, tag="zrow", bufs=3)
            nc.scalar.copy(zrow[:, 0:64], ps64[:, 0:64])
            nc.scalar.copy(zrow[:, 64:576], ps512)
            nc.sync.dma_start(out=z0w_dr[bass.ts(j, P), :], in_=zrow)

        for i in range(NQT):
            qs = bass.ts(i, P)
            csp = ps_c()
            nc.tensor.matmul(csp[:, 0:64], lhsT=xq[:, qs], rhs=W["CzA0"],
                             start=True, stop=False)
            nc.tensor.matmul(csp[:, 0:64], lhsT=ones_row[0:1, qs],
                             rhs=W["CzB0"], start=False, stop=True)
            cwp = ps_gram()
            nc.tensor.matmul(cwp, lhsT=xyzq[:, qs], rhs=W["rhs_cwA"],
                             start=True, stop=False)
            nc.tensor.matmul(cwp, lhsT=ones_row[0:1, qs], rhs=W["rhs_cwB"],
                             start=False, stop=True)
            rsp = ps_r()
            nc.tensor.matmul(rsp[:, 0:64], lhsT=xq[:, qs], rhs=W["WrT0"],
                             start=True, stop=True)

            km = gather_kmax(z0w_dr[:, :], 576, idx0_all, i)
            nc.vector.tensor_add(Wall[:, i, :], km[:, 64:576], cwp)
            nc.vector.tensor_scalar_max(Wall[:, i, :], Wall[:, i, :], 0.0)
            x1t = work.tile([128, 256], dt.float32, tag="xout", name="xout", bufs=3)
            nc.vector.tensor_add(x1t[:, 0:64], km[:, 0:64], csp[:, 0:64])
            nc.vector.tensor_scalar_max(x1t[:, 0:64], x1t[:, 0:64], 0.0)
            nc.vector.tensor_mul(x1t[:, 0:64], x1t[:, 0:64], Wall[:, i, 0:64])
            nc.vector.tensor_add(x1t[:, 0:64], x1t[:, 0:64], rsp[:, 0:64])
            nc.sync.dma_start(out=agin[1][bass.ts(i, P), :], in_=x1t[:, 0:64])

        # ===== layers 1..3 =====
        def exchange_transpose(l, XlT_tile, C):
            nc.gpsimd.collective_compute(
                "AllGather", mybir.AluOpType.bypass, replica_groups=PAIRS,
                ins=[agin[l][:, :].opt()], outs=[agout[l][:, :].opt()])
            for j in range(NJT):
                xr = work.tile([128, 128], dt.float32, tag="xrow", bufs=3)
                nc.sync.dma_start(out=xr[:, 0:C], in_=agout[l][bass.ts(j, P), :])
                pst = ps_tr()
                nc.tensor.transpose(pst[0:C, :], xr[:, 0:C], ident)
                nc.scalar.copy(XlT_tile[0:C, bass.ts(j, P)], pst[0:C, :])

        def layer(l, XlT_ap, C, C2, woff, keep_out=False):
            idx_all = consts.tile([128, NQT, 24], dt.uint32, tag="idxl", name="idxl", bufs=2)
            nc.scalar.copy(Xq[l][:, :], XlT_ap[:, bass.ds(qoff, NQ)])
            XqT_ap = Xq[l][:, :]
            compute_negxx(XlT_ap, C)
            gram_topk(XlT_ap, XqT_ap, idx_all)
            for j in range(NJT):
                psz = ps_z()
                nc.tensor.matmul(psz[:, 0:C2], lhsT=XlT_ap[:, bass.ts(j, P)],
                                 rhs=W[f"Az{l}"], start=True, stop=True)
                zrow = work.tile([128, 576], dt.float32, tag="zrow", bufs=3)
                nc.scalar.copy(zrow[:, 0:C2], psz[:, 0:C2])
                nc.sync.dma_start(out=z_dr[l][bass.ts(j, P), :], in_=zrow[:, 0:C2])
            outs = []
            for i in range(NQT):
                qs = bass.ts(i, P)
                csp = ps_c()
                nc.tensor.matmul(csp[:, 0:C2], lhsT=XqT_ap[:, qs],
                                 rhs=W[f"CzA{l}"], start=True, stop=False)
                nc.tensor.matmul(csp[:, 0:C2], lhsT=ones_row[0:1, qs],
                                 rhs=W[f"CzB{l}"], start=False, stop=True)
                rsp = ps_r()
                nc.tensor.matmul(rsp[:, 0:C2], lhsT=XqT_ap[:, qs],
                                 rhs=W[f"WrT{l}"], start=True, stop=True)
                km = gather_kmax(z_dr[l][:, :], C2, idx_all, i)
                xt = (x4keep[i] if keep_out else
                      work.tile([128, 256], dt.float32, tag="xout", name="xout", bufs=3))
                nc.vector.tensor_add(xt[:, 0:C2], km[:, 0:C2], csp[:, 0:C2])
                nc.vector.tensor_scalar_max(xt[:, 0:C2], xt[:, 0:C2], 0.0)
                nc.vector.tensor_mul(xt[:, 0:C2], xt[:, 0:C2], Wall[:, i, woff:woff + C2])
                nc.vector.tensor_add(xt[:, 0:C2], xt[:, 0:C2], rsp[:, 0:C2])
                outs.append(xt)
                if l < 3:
                    nc.sync.dma_start(out=agin[l + 1][bass.ts(i, P), :], in_=xt[:, 0:C2])
            return outs

        exchange_transpose(1, X1T, 64)
        layer(1, X1T[:, :], 64, 64, 64)
        exchange_transpose(2, X2T, 64)
        layer(2, X2T[:, :], 64, 128, 128)
        exchange_transpose(3, X3T, 128)
        x4tiles = layer(3, X3T[:, :], 128, 256, 256, keep_out=True)

        for i in range(NQT):
            for hc in range(2):
                pst = ps_tr()
                nc.tensor.transpose(pst, x4tiles[i][:, hc * 128:(hc + 1) * 128], ident)
                nc.scalar.copy(X4T[:, hc, bass.ts(i, P)], pst)

        # ===== head =====
        maxcol = work.tile([128, 8], dt.float32, tag="maxcol")
        sumcol = work.tile([128, 8], dt.float32, tag="sumcol")

        def lrelu_cols(out_ap, u_ap, ncols):
            ab = work.tile([128, 8], dt.float32, tag="lr_ab", bufs=2)
            nc.scalar.activation(ab[:, 0:ncols], u_ap, AF.Abs)
            nc.vector.tensor_scalar_mul(ab[:, 0:ncols], ab[:, 0:ncols], 0.4)
            u6 = work.tile([128, 8], dt.float32, tag="lr_u6", bufs=2)
            nc.vector.tensor_scalar_mul(u6[:, 0:ncols], u_ap, 0.6)
            nc.vector.tensor_add(out_ap, ab[:, 0:ncols], u6[:, 0:ncols])

        for g in range(8):
            wp = work.tile([128, 5, 128], dt.float32, tag="wpre_s", bufs=2)
            nc.sync.dma_start(out=wp, in_=wd["wpreT_pack"][:, :, bass.ts(g, 128)])
            accA = work.tile([128, 2], dt.float32, tag="accA", bufs=2)
            accB = work.tile([128, 2], dt.float32, tag="accB", bufs=2)
            hmax = work.tile([128, 2], dt.float32, tag="hmax", bufs=2)
            for half in range(2):
                ph = ps_gram()
                hs = bass.ts(half, 512)
                nc.tensor.matmul(ph, lhsT=wp[0:64, 0, :], rhs=Xq[1][:, hs],
                                 start=True, stop=False)
                nc.tensor.matmul(ph, lhsT=wp[0:64, 1, :], rhs=Xq[2][:, hs],
                                 start=False, stop=False)
                nc.tensor.matmul(ph, lhsT=wp[:, 2, :], rhs=Xq[3][:, hs],
                                 start=False, stop=False)
                nc.tensor.matmul(ph, lhsT=wp[:, 3, :], rhs=X4T[:, 0, hs],
                                 start=False, stop=False)
                nc.tensor.matmul(ph, lhsT=wp[:, 4, :], rhs=X4T[:, 1, hs],
                                 start=False, stop=True)
                scr = work.tile([128, 512], dt.float32, tag="hscr", bufs=2)
                nc.scalar.activation(scr, ph, AF.Identity,
                                     scale=W["spre_pack"][:, g:g + 1],
                                     bias=W["bpre_pack"][:, g:g + 1],
                                     accum_out=accB[:, half:half + 1])
                nc.vector.reduce_max(hmax[:, half:half + 1], scr, axis=mybir.AxisListType.X)
                scr2 = work.tile([128, 512], dt.float32, tag="hscr2", bufs=2)
                nc.scalar.activation(scr2, ph, AF.Abs,
                                     scale=W["spre_pack"][:, g:g + 1],
                                     bias=W["bpre_pack"][:, g:g + 1],
                                     accum_out=accA[:, half:half + 1])
            mu = work.tile([128, 1], dt.float32, tag="mu", bufs=2)
            nc.vector.tensor_tensor(out=mu, in0=hmax[:, 0:1], in1=hmax[:, 1:2],
                                    op=mybir.AluOpType.max)
            lrelu_cols(maxcol[:, g:g + 1], mu, 1)
            sA = work.tile([128, 2], dt.float32, tag="sA", bufs=2)
            nc.vector.tensor_add(sA[:, 0:1], accA[:, 0:1], accA[:, 1:2])
            nc.vector.tensor_add(sA[:, 1:2], accB[:, 0:1], accB[:, 1:2])
            nc.vector.tensor_scalar_mul(sA[:, 0:1], sA[:, 0:1], 0.4)
            nc.vector.tensor_scalar_mul(sA[:, 1:2], sA[:, 1:2], 0.6)
            nc.vector.tensor_add(sumcol[:, g:g + 1], sA[:, 0:1], sA[:, 1:2])
        pk = work.tile([128, 16], dt.float32, tag="packp")
        nc.vector.tensor_copy(pk[:, 0:8], maxcol)
        nc.vector.tensor_copy(pk[:, 8:16], sumcol)
        nc.sync.dma_start(out=pagin[:, :], in_=pk)
        nc.gpsimd.collective_compute(
            "AllGather", mybir.AluOpType.bypass, replica_groups=PAIRS,
            ins=[pagin[:, :].opt()], outs=[pagout[:, :].opt()])
        pboth = work.tile([128, 2, 16], dt.float32, tag="pboth")
        nc.sync.dma_start(out=pboth, in_=pagout[:, :].rearrange("(r p) c -> p r c", p=128))
        hh = work.tile([128, 16], dt.float32, tag="hh")
        nc.vector.tensor_tensor(out=hh[:, 0:8], in0=pboth[:, 0, 0:8], in1=pboth[:, 1, 0:8],
                                op=mybir.AluOpType.max)
        nc.vector.tensor_add(hh[:, 8:16], pboth[:, 0, 8:16], pboth[:, 1, 8:16])

        o1 = work.tile([128, 4], dt.float32, tag="o1")
        for gpass in range(2):
            pa = ps_gram()
            pb = ps_gram()
            for j in range(16):
                w1c = work.tile([128, 256], dt.float32, tag="w1_s", bufs=3)
                nc.sync.dma_start(out=w1c,
                                  in_=wd["w1T_pack"][:, j, bass.ts(gpass, 256)])
                nc.tensor.matmul(pa[:, 0:1], lhsT=w1c[:, 0:128], rhs=hh[:, j:j + 1],
                                 start=(j == 0), stop=(j == 15))
                nc.tensor.matmul(pb[:, 0:1], lhsT=w1c[:, 128:256], rhs=hh[:, j:j + 1],
                                 start=(j == 0), stop=(j == 15))
            for gi, ps in ((0, pa), (1, pb)):
                g = gpass * 2 + gi
                u1 = work.tile([128, 1], dt.float32, tag="u1", bufs=2)
                nc.scalar.activation(u1, ps[:, 0:1], AF.Identity,
                                     scale=W["s1_pack"][:, g:g + 1],
                                     bias=W["b1_pack"][:, g:g + 1])
                lrelu_cols(o1[:, g:g + 1], u1, 1)
        o2 = work.tile([128, 2], dt.float32, tag="o2")
        for g in range(2):
            ps = ps_small()
            for j in range(4):
                nc.tensor.matmul(ps[:, 0:1], lhsT=W["w2T_pack"][:, j, bass.ts(g, 128)],
                                 rhs=o1[:, j:j + 1], start=(j == 0), stop=(j == 3))
            u2 = work.tile([128, 1], dt.float32, tag="u2", bufs=2)
            nc.scalar.activation(u2, ps[:, 0:1], AF.Identity,
                                 scale=W["s2_pack"][:, g:g + 1],
                                 bias=W["b2_pack"][:, g:g + 1])
            lrelu_cols(o2[:, g:g + 1], u2, 1)
        o3 = work.tile([40, 1], dt.float32, tag="o3")
        ps3 = ps_small()
        for j in range(2):
            nc.tensor.matmul(ps3[0:40, 0:1], lhsT=W["w3T_pack"][:, j, 0:40],
                             rhs=o2[:, j:j + 1], start=(j == 0), stop=(j == 1))
        nc.vector.tensor_add(o3, ps3[0:40, 0:1], W["b3_col"][:, 0:1])
        nc.sync.dma_start(out=logits_d[:, :], in_=o3)

        for p in (work, psum, dram, feat, consts):
            p.release()

    nc.compile()
    return nc


# ======================================================================
# host-side weight fusion / packing
# ======================================================================
def pack_weights(params):
    w = {}
    sc = 1.0 / np.sqrt(1.0 + BN_EPS)
    rhs_zw, rhs_cw = [], []
    for l, C2 in enumerate(CH):
        p = params[f'layer{l}']
        wc, gc, bc = [np.asarray(p[k], np.float32) for k in ('wc', 'gc', 'bc')]
        ww, gw, bw = [np.asarray(p[k], np.float32) for k in ('ww', 'gw', 'bw')]
        wr = np.asarray(p['wr'], np.float32)
        s = gc * sc
        swl = gw * sc
        if l == 0:
            A = wc[:, 0:3]
            Bc = wc[:, 3:9]
            Aext = np.zeros((C2, 6), np.float32)
            Aext[:, 0:3] = A
            w['Az0'] = (s[:, None] * A).T.astype(np.float32)
            Cz = np.concatenate([(s[:, None] * (Bc - Aext)).T, bc[None, :]], 0)
        else:
            C = CH[l - 1]
            A = wc[:, 0:C]
            Bc = wc[:, C:2 * C]
            w[f'Az{l}'] = (s[:, None] * A).T.astype(np.float32)
            Cz = np.concatenate([(s[:, None] * (Bc - A)).T, bc[None, :]], 0)
        w[f'CzA{l}'] = np.ascontiguousarray(Cz[:-1], np.float32)
        w[f'CzB{l}'] = np.ascontiguousarray(Cz[-1:], np.float32)
        w[f'WrT{l}'] = np.ascontiguousarray(wr.T, np.float32)
        Aw, Bw = ww[:, 0:3], ww[:, 3:6]
        rhs_zw.append((swl[:, None] * Aw).T)
        rhs_cw.append(np.concatenate([(swl[:, None] * (Bw - Aw)).T, bw[None, :]], 0))
    w['rhs_zw'] = np.concatenate(rhs_zw, 1).astype(np.float32)
    cw4 = np.concatenate(rhs_cw, 1).astype(np.float32)
    w['rhs_cwA'] = np.ascontiguousarray(cw4[0:3])
    w['rhs_cwB'] = np.ascontiguousarray(cw4[3:4])

    wpreT = np.asarray(params['wpre'], np.float32).T.copy()     # [512, 1024]
    wpre_pack = np.zeros((128, 5, 1024), np.float32)
    wpre_pack[0:64, 0, :] = wpreT[0:64]
    wpre_pack[0:64, 1, :] = wpreT[64:128]
    wpre_pack[:, 2, :] = wpreT[128:256]
    wpre_pack[:, 3, :] = wpreT[256:384]
    wpre_pack[:, 4, :] = wpreT[384:512]
    w['wpreT_pack'] = wpre_pack
    w['spre_pack'] = (np.asarray(params['gpre'], np.float32) * sc).reshape(8, 128).T.copy()
    w['bpre_pack'] = np.asarray(params['bpre'], np.float32).reshape(8, 128).T.copy()

    w1 = np.asarray(params['w1'], np.float32).copy()            # [512, 2048]
    w1[:, EMB:] /= N                                            # fold mean 1/N
    w1T = w1.T.copy()                                           # [2048, 512]
    w1_pack = np.zeros((128, 16, 512), np.float32)
    for c in range(2048):
        w1_pack[c % 128, c // 128, :] = w1T[c]
    w['w1T_pack'] = w1_pack
    w['s1_pack'] = (np.asarray(params['g1'], np.float32) * sc).reshape(4, 128).T.copy()
    w['b1_pack'] = np.asarray(params['bb1'], np.float32).reshape(4, 128).T.copy()

    w2T = np.asarray(params['w2'], np.float32).T.copy()         # [512, 256]
    w2_pack = np.zeros((128, 4, 256), np.float32)
    for c in range(512):
        w2_pack[c % 128, c // 128, :] = w2T[c]
    w['w2T_pack'] = w2_pack
    s2 = np.asarray(params['g2'], np.float32) * sc
    w['s2_pack'] = s2.reshape(2, 128).T.copy()
    w['b2_pack'] = (s2 * np.asarray(params['b2'], np.float32)
                    + np.asarray(params['bb2'], np.float32)).reshape(2, 128).T.copy()

    w3T = np.asarray(params['w3'], np.float32).T.copy()         # [256, 40]
    w3_pack = np.zeros((128, 2, 40), np.float32)
    for c in range(256):
        w3_pack[c % 128, c // 128, :] = w3T[c]
    w['w3T_pack'] = w3_pack
    w['b3_col'] = np.asarray(params['b3'], np.float32).reshape(40, 1).copy()
    return w


_NC_CACHE = {}


def _get_nc():
    if "nc" not in _NC_CACHE:
        _NC_CACHE["nc"] = build_nc()
    return _NC_CACHE["nc"]


def kernel(x, params):
    x = np.asarray(x, np.float32)
    assert x.shape == (B, 6, N), x.shape
    w = pack_weights(params)
    host = {name: np.ascontiguousarray(w[name], np.float32) for name, _ in WSHAPES + WSTREAM}
    in_maps = []
    for c in range(8):
        m = dict(host)
        m['xT'] = np.ascontiguousarray(x[c // 2], np.float32)
        in_maps.append(m)
    nc = _get_nc()
    res = bass_utils.run_bass_kernel_spmd(nc, in_maps, core_ids=list(range(8)))
    out = np.stack([np.asarray(res.results[2 * b]["logits"], np.float32).reshape(40)
                    for b in range(B)])
    return out
